# revision 1
# baseline (speedup 1.0000x reference)
"""Trainium2 (8 NeuronCores) kernel for nn_ConversationLSTMStack.

Key structural facts exploited:
  - The module's hidden state is never fed back in time (h0 == zeros quirk),
    so all gate matmuls are time-parallel; only the cell state carries:
        c[t] = f[t] * c[t-1] + i[t] * g[t]
    which maps 1:1 onto the VectorEngine's tensor_tensor_scan instruction
    (op0=mult, op1=add) with time on the SBUF free axis.
  - W_hh is mathematically unused (h fed to the cells is always zero).
  - The speaker-reset quirk only zeroes layer-0's cell state; it folds into
    the scan as f_eff = f * notreset[t].

Sharding: data-parallel over batch. Each of the 8 cores gets 8 of the 64
batch rows; the small weights are replicated. Matmuls run as float32r
(full-rate fp32 PE mode), everything else in fp32.

Per-core dataflow, for each (batch, 512-step time chunk):
  DMA x^T tiles -> PE: gates = W^T.T @ x^T (PSUM, 24 gate chunks x 6 k) ->
  ACT: sigmoid/tanh with fused bias from PSUM -> DVE: u=i*g, f*=mask,
  tensor_tensor_scan -> ACT: tanh(c) -> DVE: h=o*tanh(c) -> DMA out.
Layer 0 streams h0 to a DRAM bounce buffer; layer 1 re-reads it and writes
the output shard. Host pre-transposes x to [D, B_loc, S] and re-assembles
y from [B_loc, H, S], so the device never transposes anything.
"""
import os
import sys
import types

import numpy as np

# run_bass_kernel_spmd(trace=True) imports antenv.axon_hooks, which may be
# missing from the image; provide a no-op fallback so stray BASS_TRACE env
# vars can't crash the run.
try:
    import antenv.axon_hooks  # noqa: F401
except Exception:
    try:
        import antenv  # noqa: F401
        _hooks = types.ModuleType("antenv.axon_hooks")
        _hook_box = [None]
        _hooks.set_axon_ntff_profile_hook = lambda h: _hook_box.__setitem__(0, h)
        _hooks.get_axon_ntff_profile_hook = lambda: _hook_box[0]
        sys.modules["antenv.axon_hooks"] = _hooks
    except Exception:
        pass

import concourse.bacc as bacc
import concourse.mybir as mybir
import concourse.tile as tile
from concourse.bass_utils import run_bass_kernel_spmd

F32 = mybir.dt.float32
F32R = mybir.dt.float32r
AF = mybir.ActivationFunctionType
ALU = mybir.AluOpType

N_CORES = 8
B, S, D, H = 64, 2048, 768, 768
B_LOC = B // N_CORES
TC = 512

_cache = {"nc": None}
last_exec_time_ns = None


def _build_nc(B_loc=B_LOC, S=S, D=D, H=H, TC=TC,
              x_bufs=2, g_bufs=2, c_bufs=2, h_bufs=3, p_bufs=2):
    KD = D // 128          # contraction chunks
    M4 = 4 * H // 128      # gate-dim chunks (24)
    HK = H // 128          # hidden chunks (6)
    NT = S // TC           # time chunks per batch
    G4 = 4 * H

    nc = bacc.Bacc("TRN2", target_bir_lowering=False, debug=False)
    xt = nc.dram_tensor("xt", [D, B_loc, S], F32, kind="ExternalInput")
    w0t = nc.dram_tensor("w0t", [D, G4], F32, kind="ExternalInput")
    w1t = nc.dram_tensor("w1t", [H, G4], F32, kind="ExternalInput")
    b0 = nc.dram_tensor("b0", [128, M4], F32, kind="ExternalInput")
    b1 = nc.dram_tensor("b1", [128, M4], F32, kind="ExternalInput")
    mask = nc.dram_tensor("mask", [128, S], F32, kind="ExternalInput")
    y = nc.dram_tensor("y", [B_loc, H, S], F32, kind="ExternalOutput")
    hcfin = nc.dram_tensor("hcfin", [2, 2, B_loc, H], F32, kind="ExternalOutput")

    with tile.TileContext(nc) as tc:
        with (
            tc.tile_pool(name="wp", bufs=1) as wp,
            tc.tile_pool(name="xp", bufs=x_bufs) as xp,
            tc.tile_pool(name="gp", bufs=g_bufs) as gp,
            tc.tile_pool(name="cp", bufs=c_bufs) as cp,
            tc.tile_pool(name="hp", bufs=h_bufs) as hp,
            tc.tile_pool(name="mp", bufs=1) as mp,
            tc.tile_pool(name="pp", bufs=p_bufs, space="PSUM") as pp,
            tc.tile_pool(name="dp", bufs=1, space="DRAM") as dp,
        ):
            mask_t = mp.tile([128, S], F32, tag="mask")
            nc.sync.dma_start(mask_t[:], mask[:])
            h0buf = [dp.tile([H, S], F32, tag=f"h0b{b}", name=f"h0b{b}")
                     for b in range(B_loc)]

            for layer in range(2):
                wdram = w0t if layer == 0 else w1t
                bdram = b0 if layer == 0 else b1
                bias_t = mp.tile([128, M4], F32, tag=f"bias{layer}")
                nc.sync.dma_start(bias_t[:], bdram[:])
                w_sb = []
                for k in range(KD):
                    wt_t = wp.tile([128, G4], F32R, tag=f"w{k}")
                    nc.sync.dma_start(
                        wt_t[:], wdram[128 * k:128 * (k + 1), :].bitcast(F32R))
                    w_sb.append(wt_t)

                for b in range(B_loc):
                    carry = [None] * HK
                    for t in range(NT):
                        tsl = slice(t * TC, (t + 1) * TC)
                        xts = []
                        for k in range(KD):
                            xt_t = xp.tile([128, TC], F32R, tag=f"x{k}")
                            if layer == 0:
                                src = xt[128 * k:128 * (k + 1), b, tsl]
                            else:
                                src = h0buf[b][128 * k:128 * (k + 1), tsl]
                            nc.sync.dma_start(xt_t[:], src.bitcast(F32R))
                            xts.append(xt_t)
                        for hk in range(HK):
                            ps_tiles = []
                            for gi, m in enumerate(
                                    (hk, HK + hk, 2 * HK + hk, 3 * HK + hk)):
                                pt = pp.tile([128, TC], F32, tag=f"p{gi}")
                                for k in range(KD):
                                    nc.tensor.matmul(
                                        pt[:],
                                        w_sb[k][:, 128 * m:128 * (m + 1)],
                                        xts[k][:],
                                        start=(k == 0), stop=(k == KD - 1))
                                ps_tiles.append(pt)
                            i_t = gp.tile([128, TC], F32, tag="gi")
                            nc.scalar.activation(i_t[:], ps_tiles[0][:], AF.Sigmoid,
                                                 bias=bias_t[:, hk:hk + 1])
                            f_t = gp.tile([128, TC], F32, tag="gf")
                            nc.scalar.activation(f_t[:], ps_tiles[1][:], AF.Sigmoid,
                                                 bias=bias_t[:, HK + hk:HK + hk + 1])
                            g_t = gp.tile([128, TC], F32, tag="gg")
                            nc.scalar.activation(g_t[:], ps_tiles[2][:], AF.Tanh,
                                                 bias=bias_t[:, 2 * HK + hk:2 * HK + hk + 1])
                            o_t = gp.tile([128, TC], F32, tag="go")
                            nc.scalar.activation(o_t[:], ps_tiles[3][:], AF.Sigmoid,
                                                 bias=bias_t[:, 3 * HK + hk:3 * HK + hk + 1])
                            if layer == 0:
                                nc.vector.tensor_mul(f_t[:], f_t[:], mask_t[:, tsl])
                            u_t = gp.tile([128, TC], F32, tag="gu")
                            nc.vector.tensor_mul(u_t[:], i_t[:], g_t[:])
                            c_t = cp.tile([128, TC], F32, tag=f"c{hk}")
                            init = 0.0 if t == 0 else carry[hk][:, TC - 1:TC]
                            nc.vector.tensor_tensor_scan(c_t[:], f_t[:], u_t[:], init,
                                                         op0=ALU.mult, op1=ALU.add)
                            carry[hk] = c_t
                            th_t = gp.tile([128, TC], F32, tag="gth")
                            nc.scalar.activation(th_t[:], c_t[:], AF.Tanh)
                            h_t = hp.tile([128, TC], F32, tag="h")
                            nc.vector.tensor_mul(h_t[:], o_t[:], th_t[:])
                            hsl = slice(128 * hk, 128 * (hk + 1))
                            if layer == 0:
                                nc.sync.dma_start(h0buf[b][hsl, tsl], h_t[:])
                            else:
                                nc.sync.dma_start(y[b, hsl, tsl], h_t[:])
                            if t == NT - 1:
                                nc.sync.dma_start(hcfin[0, layer, b, hsl],
                                                  h_t[:, TC - 1:TC])
                                nc.sync.dma_start(hcfin[1, layer, b, hsl],
                                                  c_t[:, TC - 1:TC])
    nc.compile()
    return nc


def kernel(x, W_ih, W_hh, b_ih, b_hh, participants):
    global last_exec_time_ns
    x = np.asarray(x, dtype=np.float32)
    W_ih = np.asarray(W_ih, dtype=np.float32)
    b_ih = np.asarray(b_ih, dtype=np.float32)
    b_hh = np.asarray(b_hh, dtype=np.float32)
    participants = np.asarray(participants)

    if _cache["nc"] is None:
        _cache["nc"] = _build_nc()
    nc = _cache["nc"]

    # host-side prep (sharding + layout)
    reset = np.empty(S, dtype=bool)
    reset[0] = True
    reset[1:] = participants[1:] != participants[:-1]
    notr = (~reset).astype(np.float32)
    mask = np.ascontiguousarray(np.broadcast_to(notr, (128, S)))
    w0t = np.ascontiguousarray(W_ih[0].T)
    w1t = np.ascontiguousarray(W_ih[1].T)
    b0 = np.ascontiguousarray((b_ih[0] + b_hh[0]).reshape(-1, 128).T)
    b1 = np.ascontiguousarray((b_ih[1] + b_hh[1]).reshape(-1, 128).T)
    in_maps = []
    for c in range(N_CORES):
        xs = x[c * B_LOC:(c + 1) * B_LOC]                   # [B_loc, S, D]
        xtc = np.ascontiguousarray(xs.transpose(2, 0, 1))   # [D, B_loc, S]
        in_maps.append({"xt": xtc, "w0t": w0t, "w1t": w1t,
                        "b0": b0, "b1": b1, "mask": mask})

    trace = bool(int(os.environ.get("LSTM_KERNEL_TRACE", "0")))
    res = run_bass_kernel_spmd(nc, in_maps, core_ids=list(range(N_CORES)),
                               trace=trace)
    last_exec_time_ns = res.exec_time_ns

    outputs = np.empty((S, B, H), dtype=np.float32)
    h_fin = np.empty((2, B, H), dtype=np.float32)
    c_fin = np.empty((2, B, H), dtype=np.float32)
    for c in range(N_CORES):
        yc = res.results[c]["y"]                            # [B_loc, H, S]
        outputs[:, c * B_LOC:(c + 1) * B_LOC, :] = yc.transpose(2, 0, 1)
        hc = res.results[c]["hcfin"]                        # [2, 2, B_loc, H]
        h_fin[:, c * B_LOC:(c + 1) * B_LOC, :] = hc[0]
        c_fin[:, c * B_LOC:(c + 1) * B_LOC, :] = hc[1]
    return outputs, h_fin, c_fin


# revision 2
# speedup vs baseline: 1.0245x; 1.0245x over previous
"""Trainium2 (8 NeuronCores) kernel for nn_ConversationLSTMStack.

Key structural facts exploited:
  - The module's hidden state is never fed back in time (h0 == zeros quirk),
    so all gate matmuls are time-parallel; only the cell state carries:
        c[t] = f[t] * c[t-1] + i[t] * g[t]
    which maps 1:1 onto the VectorEngine's tensor_tensor_scan instruction
    (op0=mult, op1=add) with time on the SBUF free axis.
  - W_hh is mathematically unused (the h fed to every cell is zero).
  - The speaker-reset quirk only zeroes layer-0's cell state; it folds into
    the scan as f_eff = f * notreset[t].

Sharding: data-parallel over batch. Each of the 8 cores gets 8 of the 64
batch rows; the small weights are replicated. Matmuls run as float32r
(full-rate fp32 PE mode, ~74 TFLOP/s/core measured), everything else fp32.

Per-core dataflow, for each (batch, 512-step time chunk):
  DMA x^T tiles -> PE: gates = W^T.T @ x^T (PSUM, 24 gate chunks x 6 k) ->
  ACT: sigmoid/tanh with fused bias from PSUM -> DVE: u=i*g, f*=mask,
  tensor_tensor_scan -> ACT: tanh(c) -> DVE: h=o*tanh(c) -> DMA out.
Layer 0 streams h0 to a per-batch DRAM bounce buffer; layer 1 re-reads it
and writes the output shard. The host pre-transposes x to [D, B_loc, S] and
re-assembles y from [B_loc, H, S], so the device never transposes.

Weights are split into 18 sub-tiles per layer: (k, j) of [128, 1024], where
j indexes a "pair group" covering the 4 gates of hidden chunks 2j, 2j+1
(host pre-permutes weight columns so each group is contiguous). The first
chunk of each phase accumulates k-OUTER across an 8-bank PSUM group so the
PE starts as soon as the first weight sub-tile lands; layer-1's j=0 tiles
are prefetched during phase 1 to hide the phase boundary.
"""
import os
import sys
import types

import numpy as np

# run_bass_kernel_spmd(trace=True) imports antenv.axon_hooks, which is
# missing from this image; provide a no-op fallback so a stray BASS_TRACE
# env var cannot crash the run (trace=False never touches it).
try:
    import antenv.axon_hooks  # noqa: F401
except Exception:
    try:
        import antenv  # noqa: F401
        _hooks = types.ModuleType("antenv.axon_hooks")
        _hook_box = [None]
        _hooks.set_axon_ntff_profile_hook = lambda h: _hook_box.__setitem__(0, h)
        _hooks.get_axon_ntff_profile_hook = lambda: _hook_box[0]
        sys.modules["antenv.axon_hooks"] = _hooks
    except Exception:
        pass

import concourse.bacc as bacc
import concourse.mybir as mybir
import concourse.tile as tile
from concourse.bass_utils import run_bass_kernel_spmd
from concourse.tile_rust import add_dep_helper

F32 = mybir.dt.float32
F32R = mybir.dt.float32r
AF = mybir.ActivationFunctionType
ALU = mybir.AluOpType

N_CORES = 8
B, S, D, H = 64, 2048, 768, 768
B_LOC = B // N_CORES
TC = 512

_cache = {"nc": None}
last_exec_time_ns = None


def _perm_cols():
    """Weight-column chunk permutation: group j holds the 4 gates of hidden
    chunks 2j, 2j+1 contiguously."""
    p = []
    for j in range(3):
        for gi in range(4):
            for r in range(2):
                p.append(gi * 6 + (2 * j + r))
    return np.array(p)


def _build_nc(B_loc=B_LOC, S=S, D=D, H=H, TC=TC,
              x_bufs=3, g_bufs=4, c_bufs=2, h_bufs=4, prefetch_j=(0,)):
    KD = D // 128
    HK = H // 128
    NJ = HK // 2
    NT = S // TC
    G4 = 4 * H

    nc = bacc.Bacc("TRN2", target_bir_lowering=False, debug=False)
    xt = nc.dram_tensor("xt", [D, B_loc, S], F32, kind="ExternalInput")
    w0t = nc.dram_tensor("w0t", [D, G4], F32, kind="ExternalInput")
    w1t = nc.dram_tensor("w1t", [H, G4], F32, kind="ExternalInput")
    b0 = nc.dram_tensor("b0", [128, 24], F32, kind="ExternalInput")
    b1 = nc.dram_tensor("b1", [128, 24], F32, kind="ExternalInput")
    mask = nc.dram_tensor("mask", [128, S], F32, kind="ExternalInput")
    y = nc.dram_tensor("y", [B_loc, H, S], F32, kind="ExternalOutput")
    hcfin = nc.dram_tensor("hcfin", [2, 2, B_loc, H], F32, kind="ExternalOutput")

    with tile.TileContext(nc) as tc:
        with (
            tc.tile_pool(name="wp", bufs=1) as wp,
            tc.tile_pool(name="xp", bufs=x_bufs) as xp,
            tc.tile_pool(name="gp", bufs=g_bufs) as gp,
            tc.tile_pool(name="cp", bufs=c_bufs) as cp,
            tc.tile_pool(name="hp", bufs=h_bufs) as hp,
            tc.tile_pool(name="mp", bufs=1) as mp,
            tc.tile_pool(name="pp", bufs=1, space="PSUM") as pp,
            tc.tile_pool(name="dp", bufs=1, space="DRAM") as dp,
        ):
            mask_t = mp.tile([128, S], F32, tag="mask")
            mask_dma = nc.sync.dma_start(mask_t[:], mask[:])
            h0buf = [dp.tile([H, S], F32, tag=f"h0b{b}", name=f"h0b{b}")
                     for b in range(B_loc)]

            w_sb = {}
            w_dma = {}
            bias_t = {}
            stage = {}

            def load_w_tile(layer, k, j):
                wdram = w0t if layer == 0 else w1t
                tag = (f"w1p_{k}_{j}" if (layer == 1 and j in prefetch_j)
                       else f"w_{k}_{j}")
                t = wp.tile([128, 1024], F32R, tag=tag, name=f"w{layer}_{k}_{j}")
                w_dma[(layer, k, j)] = nc.sync.dma_start(
                    t[:], wdram[128 * k:128 * (k + 1),
                                1024 * j:1024 * (j + 1)].bitcast(F32R))
                return t

            def load_layer_weights(layer, js):
                for j in js:
                    for k in range(KD):
                        w_sb[(layer, k, j)] = load_w_tile(layer, k, j)

            def epilogue_pair(layer, b, t, j, psums, tsl):
                """psums: 8 tiles, local index l = gi*2 + r."""
                if t == NT - 1 and j == 0:
                    stage["h"] = mp.tile([128, HK], F32, tag="stgh",
                                         name=f"stgh_{layer}_{b}", bufs=2)
                    stage["c"] = mp.tile([128, HK], F32, tag="stgc",
                                         name=f"stgc_{layer}_{b}", bufs=2)
                for r in range(2):
                    hk = 2 * j + r
                    bt = bias_t[layer]
                    gs = {}
                    for gi, tag, func in ((0, "gi", AF.Sigmoid),
                                          (1, "gf", AF.Sigmoid),
                                          (2, "gg", AF.Tanh),
                                          (3, "go", AF.Sigmoid)):
                        g_t = gp.tile([128, TC], F32, tag=tag,
                                      name=f"g{tag}_{layer}_{b}_{t}_{hk}")
                        col = (hk // 2) * 8 + gi * 2 + hk % 2
                        nc.scalar.activation(g_t[:], psums[gi * 2 + r][:], func,
                                             bias=bt[:, col:col + 1])
                        gs[gi] = g_t
                    if layer == 0:
                        nc.vector.tensor_mul(gs[1][:], gs[1][:], mask_t[:, tsl])
                    # u = i*g, in place on i's tile
                    nc.vector.tensor_mul(gs[0][:], gs[0][:], gs[2][:])
                    c_t = cp.tile([128, TC], F32, tag=f"c{hk}",
                                  name=f"c_{layer}_{b}_{t}_{hk}")
                    init = 0.0 if t == 0 else carry[hk][:, TC - 1:TC]
                    nc.vector.tensor_tensor_scan(c_t[:], gs[1][:], gs[0][:], init,
                                                 op0=ALU.mult, op1=ALU.add)
                    carry[hk] = c_t
                    th_t = gp.tile([128, TC], F32, tag="gg",
                                   name=f"th_{layer}_{b}_{t}_{hk}")
                    nc.scalar.activation(th_t[:], c_t[:], AF.Tanh)
                    h_t = hp.tile([128, TC], F32, tag="h",
                                  name=f"h_{layer}_{b}_{t}_{hk}")
                    nc.vector.tensor_mul(h_t[:], gs[3][:], th_t[:])
                    hsl = slice(128 * hk, 128 * (hk + 1))
                    if layer == 0:
                        nc.sync.dma_start(h0buf[b][hsl, tsl], h_t[:])
                    else:
                        nc.sync.dma_start(y[b, hsl, tsl], h_t[:])
                    if t == NT - 1:
                        nc.vector.tensor_copy(stage["h"][:, hk:hk + 1],
                                              h_t[:, TC - 1:TC])
                        nc.vector.tensor_copy(stage["c"][:, hk:hk + 1],
                                              c_t[:, TC - 1:TC])
                        if hk == HK - 1:
                            for si, nm in ((0, "h"), (1, "c")):
                                dst = hcfin[si, layer, b].rearrange(
                                    "(a p) -> p a", p=128)
                                nc.sync.dma_start(dst, stage[nm][:])

            for layer in range(2):
                bdram = b0 if layer == 0 else b1
                bias_t[layer] = mp.tile([128, 24], F32, tag=f"bias{layer}",
                                        name=f"bias{layer}")
                nc.sync.dma_start(bias_t[layer][:], bdram[:])
                if layer == 0:
                    load_layer_weights(0, range(NJ))
                    # stagger startup DMA bandwidth: j depends on j-1's last
                    # k-tile; mask waits for the critical j=0 set
                    for j in range(1, NJ):
                        for k in range(KD):
                            add_dep_helper(
                                w_dma[(0, k, j)].ins,
                                w_dma[(0, KD - 1, j - 1)].ins,
                                reason="stagger startup w DMAs")
                    add_dep_helper(mask_dma.ins, w_dma[(0, KD - 1, 0)].ins,
                                   reason="mask after critical w")
                else:
                    load_layer_weights(1, [j for j in range(NJ)
                                           if j not in prefetch_j])

                for b in range(B_loc):
                    carry = [None] * HK
                    for t in range(NT):
                        tsl = slice(t * TC, (t + 1) * TC)
                        xts = []
                        for k in range(KD):
                            xt_t = xp.tile([128, TC], F32R, tag=f"x{k}",
                                           name=f"x_{layer}_{b}_{t}_{k}")
                            if layer == 0:
                                src = xt[128 * k:128 * (k + 1), b, tsl]
                            else:
                                src = h0buf[b][128 * k:128 * (k + 1), tsl]
                            nc.sync.dma_start(xt_t[:], src.bitcast(F32R))
                            xts.append(xt_t)
                        first_chunk = (b == 0 and t == 0)
                        for j in range(NJ):
                            psums = [pp.tile([128, TC], F32, tag=f"p{l}",
                                             name=f"ps_{layer}_{b}_{t}_{j}_{l}")
                                     for l in range(8)]
                            if first_chunk:
                                # k-outer: PE starts on the first w sub-tile
                                for k in range(KD):
                                    wt = w_sb[(layer, k, j)]
                                    for l in range(8):
                                        nc.tensor.matmul(
                                            psums[l][:],
                                            wt[:, 128 * l:128 * (l + 1)],
                                            xts[k][:],
                                            start=(k == 0), stop=(k == KD - 1))
                            else:
                                for l in range(8):
                                    for k in range(KD):
                                        wt = w_sb[(layer, k, j)]
                                        nc.tensor.matmul(
                                            psums[l][:],
                                            wt[:, 128 * l:128 * (l + 1)],
                                            xts[k][:],
                                            start=(k == 0), stop=(k == KD - 1))
                            epilogue_pair(layer, b, t, j, psums, tsl)
                    if layer == 0 and b == min(2, B_loc - 1):
                        load_layer_weights(1, list(prefetch_j))
    nc.compile()
    return nc


def kernel(x, W_ih, W_hh, b_ih, b_hh, participants):
    global last_exec_time_ns
    x = np.asarray(x, dtype=np.float32)
    W_ih = np.asarray(W_ih, dtype=np.float32)
    b_ih = np.asarray(b_ih, dtype=np.float32)
    b_hh = np.asarray(b_hh, dtype=np.float32)
    participants = np.asarray(participants)

    if _cache["nc"] is None:
        _cache["nc"] = _build_nc()
    nc = _cache["nc"]

    reset = np.empty(S, dtype=bool)
    reset[0] = True
    reset[1:] = participants[1:] != participants[:-1]
    notr = (~reset).astype(np.float32)
    mask = np.ascontiguousarray(np.broadcast_to(notr, (128, S)))
    p = _perm_cols()
    idx = (p[:, None] * 128 + np.arange(128)[None, :]).ravel()
    w0t = np.ascontiguousarray(W_ih[0].T[:, idx])
    w1t = np.ascontiguousarray(W_ih[1].T[:, idx])
    b0 = np.ascontiguousarray((b_ih[0] + b_hh[0]).reshape(24, 128)[p].T)
    b1 = np.ascontiguousarray((b_ih[1] + b_hh[1]).reshape(24, 128)[p].T)
    in_maps = []
    for c in range(N_CORES):
        xs = x[c * B_LOC:(c + 1) * B_LOC]                   # [B_loc, S, D]
        xtc = np.ascontiguousarray(xs.transpose(2, 0, 1))   # [D, B_loc, S]
        in_maps.append({"xt": xtc, "w0t": w0t, "w1t": w1t,
                        "b0": b0, "b1": b1, "mask": mask})

    trace = bool(int(os.environ.get("LSTM_KERNEL_TRACE", "0")))
    res = run_bass_kernel_spmd(nc, in_maps, core_ids=list(range(N_CORES)),
                               trace=trace)
    last_exec_time_ns = res.exec_time_ns

    outputs = np.empty((S, B, H), dtype=np.float32)
    h_fin = np.empty((2, B, H), dtype=np.float32)
    c_fin = np.empty((2, B, H), dtype=np.float32)
    for c in range(N_CORES):
        yc = res.results[c]["y"]                            # [B_loc, H, S]
        outputs[:, c * B_LOC:(c + 1) * B_LOC, :] = yc.transpose(2, 0, 1)
        hc = res.results[c]["hcfin"]                        # [2, 2, B_loc, H]
        h_fin[:, c * B_LOC:(c + 1) * B_LOC, :] = hc[0]
        c_fin[:, c * B_LOC:(c + 1) * B_LOC, :] = hc[1]
    return outputs, h_fin, c_fin
